# revision 9
# baseline (speedup 1.0000x reference)
"""Trainium2 Bass kernel for nn_EnhancedGAT (3-layer GATv2, N=10000, E=160000).

Strategy (8 NeuronCores, SPMD):
  - Destination-partition the graph: each core owns 1250 dst nodes (padded to
    1280 = 10 windows x 128). Edges (incl. self-loops) are sorted by dst on the
    host and bucketed into (core, window); each window's edge list is padded to
    T_w tiles of 128 edges.
  - Segment softmax without segment-max (logits are tiny):
        out = (sum_e exp(l_e) * xl[src_e]) / (sum_e exp(l_e) + eps)
    one pass over edges; scatter-add done with one-hot matmuls on the PE into
    PSUM accumulators per 128-dst window.
  - Layer 1 uses a "G-trick": aggregate w-weighted one-hot @ raw x features
    (128-dim) per window, then multiply by Wl1 once per window; avoids building
    or gathering the 1024-wide xl1 table entirely.
  - Layers 2/3 gather from xl tables built locally and AllGather'd across the
    8 cores (DRAM bounce + collective).
  - One-hot tiles are generated on-chip from dst indices (iota + is_equal),
    never streamed from HBM.
"""
import os
import numpy as np

import concourse.bass as bass
import concourse.bacc as bacc
import concourse.mybir as mybir
import concourse.tile as tile
from concourse.bass_utils import run_bass_kernel_spmd
from concourse.masks import make_identity

F32 = mybir.dt.float32
I32 = mybir.dt.int32
AF = mybir.ActivationFunctionType
OP = mybir.AluOpType

NC_CORES = 8
N = 10000
ND = 128
ED = 32
PER = N // NC_CORES          # 1250
NPAD = 1280
W = NPAD // 128              # 10 windows
EPS = 1e-16

_DEBUG = bool(int(os.environ.get("GAT_DEBUG", "0")))
_DEBUG2 = bool(int(os.environ.get("GAT_DEBUG2", "0")))
_WLIM = int(os.environ.get("GAT_WLIM", str(W)))    # windows to emit (dev only)
_LAYERS = int(os.environ.get("GAT_LAYERS", "3"))   # dev only


# ----------------------------------------------------------------------------
# host-side prep
# ----------------------------------------------------------------------------

def _host_prep(x, edge_index, edge_attr):
    src = np.concatenate([edge_index[0], np.arange(N)]).astype(np.int64)
    dst = np.concatenate([edge_index[1], np.arange(N)]).astype(np.int64)
    ea = np.concatenate(
        [edge_attr, np.tile(edge_attr.mean(0), (N, 1))], axis=0
    ).astype(np.float32)

    core_of = dst // PER
    loc = dst % PER
    win_of = loc // 128
    dst_rel = (loc % 128).astype(np.float32)

    key = core_of * W + win_of
    order = np.argsort(key, kind="stable")
    counts = np.bincount(key[order], minlength=NC_CORES * W)
    T_w = int(np.ceil(counts.max() / 128))
    EPW = T_w * 128

    starts = np.zeros(NC_CORES * W, np.int64)
    starts[1:] = np.cumsum(counts)[:-1]

    src1 = np.zeros((NC_CORES, W, EPW), np.int32)
    src23 = np.zeros((NC_CORES, W, EPW), np.int32)
    drel = np.full((NC_CORES, W, EPW), -1.0, np.float32)
    eaT = np.zeros((NC_CORES, W, ED, EPW), np.float32)
    for c in range(NC_CORES):
        for w in range(W):
            k = int(counts[c * W + w])
            m = order[starts[c * W + w]: starts[c * W + w] + k]
            src1[c, w, :k] = src[m]
            src23[c, w, :k] = (src[m] // PER) * NPAD + (src[m] % PER)
            drel[c, w, :k] = dst_rel[m]
            eaT[c, w, :, :k] = ea[m].T
    return T_w, EPW, src1, src23, drel, eaT


def _pad_own(a, c):
    out = np.zeros((NPAD,) + a.shape[1:], a.dtype)
    out[:PER] = a[c * PER: (c + 1) * PER]
    return out


def _chunks_for_rhs(Wm):
    """[K, F] weight -> [128, (K//128)*F]: chunk k at cols [k*F:(k+1)*F]."""
    K, F = Wm.shape
    assert K % 128 == 0
    return np.ascontiguousarray(
        Wm.reshape(K // 128, 128, F).transpose(1, 0, 2).reshape(128, -1)
    )


# ----------------------------------------------------------------------------
# bass program
# ----------------------------------------------------------------------------

def _build_program(T_w):
    EPW = T_w * 128
    nc = bacc.Bacc("TRN2", target_bir_lowering=False, debug=False,
                   enable_asserts=False, num_devices=NC_CORES)

    def din(name, shape, dt=F32):
        return nc.dram_tensor(name, shape, dt, kind="ExternalInput")

    x_full = din("x_full", [N, ND])
    x_ownT = din("x_ownT", [ND, NPAD])
    src1_d = din("src1", [W * 128, T_w], I32)
    src23_d = din("src23", [W * 128, T_w], I32)
    drel_d = din("drel", [W * 128, T_w])
    eaT_d = din("eaT", [W * ED, EPW])
    iotar_d = din("iotar", [128, 128])

    Wl1_d = din("Wl1", [128, 1024])
    Wr1_d = din("Wr1", [128, 1024])
    Wres_d = din("Wres", [128, 1024])
    We1_d = din("We1", [ED, 1024])
    att1_d = din("att1b", [128, 1024])
    Wl2_d = din("Wl2c", [128, 8 * 512])
    Wr2_d = din("Wr2c", [128, 8 * 512])
    We2_d = din("We2", [ED, 512])
    att2_d = din("att2b", [128, 512])
    Wl3_d = din("Wl3", [128, 128])
    Wr3_d = din("Wr3", [128, 128])
    We3_d = din("We3", [ED, 128])
    att3_d = din("att3b", [128, 128])
    Wc1_d = din("Wc1", [128, 64])
    Wc2_d = din("Wc2", [64, 3])
    biasr1_d = din("biasr1", [1, 1024])
    const1_d = din("const1", [1, 1024])
    biasr2_d = din("biasr2", [1, 512])
    const2_d = din("const2b", [128, 128])
    biasr3_d = din("biasr3", [1, 128])
    const3_d = din("const3b", [128, 128])
    bc1_d = din("bc1", [1, 64])
    bc2_d = din("bc2", [1, 3])

    out_d = nc.dram_tensor("out_o", [NPAD, 3], F32, kind="ExternalOutput")
    if _DEBUG:
        h1_dbg = nc.dram_tensor("h1_dbg", [NPAD, 1024], F32, kind="ExternalOutput")
        h2_dbg = nc.dram_tensor("h2_dbg", [NPAD, 128], F32, kind="ExternalOutput")
        h3_dbg = nc.dram_tensor("h3_dbg", [NPAD, 128], F32, kind="ExternalOutput")
    if _DEBUG2:
        d_oh = nc.dram_tensor("d_oh", [128, 128], F32, kind="ExternalOutput")
        d_xg = nc.dram_tensor("d_xg", [128, 128], F32, kind="ExternalOutput")
        d_ps = nc.dram_tensor("d_ps", [128, 1024], F32, kind="ExternalOutput")
        d_s = nc.dram_tensor("d_s", [128, 1024], F32, kind="ExternalOutput")
        d_we = nc.dram_tensor("d_we", [128, 8], F32, kind="ExternalOutput")
        d_G = nc.dram_tensor("d_G", [128, 1032], F32, kind="ExternalOutput")
        d_gn = nc.dram_tensor("d_gn", [128, 1024], F32, kind="ExternalOutput")
        d_xr = nc.dram_tensor("d_xr", [128, 1024], F32, kind="ExternalOutput")

    with tile.TileContext(nc) as tc:
        with tc.tile_pool(name="wp", bufs=1) as wp, \
             tc.tile_pool(name="slab", bufs=1) as slab, \
             tc.tile_pool(name="io", bufs=2) as io, \
             tc.tile_pool(name="io3", bufs=3) as io3, \
             tc.tile_pool(name="io2", bufs=2) as io2, \
             tc.tile_pool(name="fat", bufs=2) as fat, \
             tc.tile_pool(name="tp1", bufs=1) as tp1, \
             tc.tile_pool(name="big", bufs=2) as big, \
             tc.tile_pool(name="psS", bufs=2, space="PSUM") as psS, \
             tc.tile_pool(name="psG", bufs=1, space="PSUM") as psG, \
             tc.tile_pool(name="psT", bufs=1, space="PSUM") as psT, \
             tc.tile_pool(name="dram", bufs=1, space="DRAM") as dr:

            # ---------- resident constants ----------
            def load(dram_t, shape, name, dt=F32):
                t = wp.tile(shape, dt, name=name, tag=name)
                nc.sync.dma_start(out=t[:], in_=dram_t.ap())
                return t

            Wl1 = load(Wl1_d, [128, 1024], "Wl1")
            Wr1 = load(Wr1_d, [128, 1024], "Wr1")
            Wres = load(Wres_d, [128, 1024], "Wres")
            We1 = load(We1_d, [ED, 1024], "We1")
            att1b = load(att1_d, [128, 1024], "att1b")
            Wl2 = load(Wl2_d, [128, 8 * 512], "Wl2")
            Wr2 = load(Wr2_d, [128, 8 * 512], "Wr2")
            We2 = load(We2_d, [ED, 512], "We2")
            att2b = load(att2_d, [128, 512], "att2b")
            Wl3 = load(Wl3_d, [128, 128], "Wl3")
            Wr3 = load(Wr3_d, [128, 128], "Wr3")
            We3 = load(We3_d, [ED, 128], "We3")
            att3b = load(att3_d, [128, 128], "att3b")
            Wc1 = load(Wc1_d, [128, 64], "Wc1")
            Wc2 = load(Wc2_d, [64, 3], "Wc2")
            biasr1 = load(biasr1_d, [1, 1024], "biasr1")
            const1 = load(const1_d, [1, 1024], "const1")
            biasr2 = load(biasr2_d, [1, 512], "biasr2")
            const2b = load(const2_d, [128, 128], "const2b")
            biasr3 = load(biasr3_d, [1, 128], "biasr3")
            const3b = load(const3_d, [128, 128], "const3b")
            bc1 = load(bc1_d, [1, 64], "bc1")
            bc2 = load(bc2_d, [1, 3], "bc2")
            iotar = load(iotar_d, [128, 128], "iotar")

            ident = wp.tile([128, 128], F32, name="ident", tag="ident")
            make_identity(nc, ident[:])
            ones1 = wp.tile([1, 128], F32, name="ones1", tag="ones1")
            nc.vector.memset(ones1[:], 1.0)

            xr2_own = slab.tile([128, W * 512], F32, name="xr2_own", tag="xr2_own")
            xr3_own = slab.tile([128, W * 128], F32, name="xr3_own", tag="xr3_own")

            xl2_bounce = dr.tile([NPAD, 512], F32, name="xl2_bounce")
            xl2_full = dr.tile([NC_CORES * NPAD, 512], F32, name="xl2_full",
                               addr_space="Shared")
            xl3_bounce = dr.tile([NPAD, 128], F32, name="xl3_bounce")
            xl3_full = dr.tile([NC_CORES * NPAD, 128], F32, name="xl3_full",
                               addr_space="Shared")

            # ---------- helpers ----------
            def window_meta(w, lidx):
                idx_w = io3.tile([128, T_w], I32, name=f"idx{lidx}_{w}", tag="idx")
                src_d = src1_d if lidx == 1 else src23_d
                nc.sync.dma_start(out=idx_w[:], in_=src_d.ap()[w * 128:(w + 1) * 128, :])
                drel_w = io3.tile([128, T_w], F32, name=f"drel{lidx}_{w}", tag="drel")
                nc.sync.dma_start(out=drel_w[:], in_=drel_d.ap()[w * 128:(w + 1) * 128, :])
                ea_w = big.tile([ED, EPW], F32, name=f"ea{lidx}_{w}", tag="ea")
                nc.sync.dma_start(out=ea_w[:], in_=eaT_d.ap()[w * ED:(w + 1) * ED, :])
                return idx_w, drel_w, ea_w

            def onehot(drel_w, t, w, lidx):
                oh = io3.tile([128, 128], F32, name=f"oh{lidx}_{w}_{t}", tag="oh")
                nc.vector.tensor_scalar(out=oh[:], in0=iotar[:],
                                        scalar1=drel_w[:, t:t + 1], scalar2=None,
                                        op0=OP.is_equal)
                ps = psT.tile([128, 128], F32, name=f"pst{lidx}_{w}_{t}", tag="scr")
                nc.tensor.transpose(out=ps[:], in_=oh[:], identity=ident[:])
                ohT = io3.tile([128, 128], F32, name=f"ohT{lidx}_{w}_{t}", tag="ohT")
                nc.scalar.copy(out=ohT[:], in_=ps[:])
                return oh, ohT

            def leaky(pt, F, name, pool):
                """s = max(v, 0.2 v); v may be PSUM or SBUF AP."""
                s = pool.tile([128, F], F32, name=name, tag=f"lk{F}")
                nc.scalar.activation(out=s[:], in_=pt, func=AF.Copy, scale=0.2)
                nc.vector.tensor_tensor(out=s[:], in0=pt, in1=s[:], op=OP.max)
                return s

            def logits_w(s, attb, H, w, t, lidx, pool):
                u = pool.tile([128, H * 128], F32, name=f"u{lidx}_{w}_{t}", tag=f"u{H}")
                nc.vector.tensor_tensor(out=u[:], in0=s[:], in1=attb[:, :H * 128], op=OP.mult)
                lg = io.tile([128, H], F32, name=f"lg{lidx}_{w}_{t}", tag="lg")
                uv = u[:].rearrange("p (h c) -> p h c", h=H) if H > 1 else u[:]
                nc.vector.tensor_reduce(out=lg[:], in_=uv, axis=mybir.AxisListType.X,
                                        op=OP.add)
                we = io.tile([128, H], F32, name=f"we{lidx}_{w}_{t}", tag="we")
                nc.scalar.activation(out=we[:], in_=lg[:], func=AF.Exp)
                return we

            def rz_from(ps_z, H, w, lidx, quarter=False):
                zt = io.tile([128, H], F32, name=f"zt{lidx}_{w}", tag="zt")
                nc.vector.tensor_scalar(out=zt[:], in0=ps_z, scalar1=EPS,
                                        scalar2=None, op0=OP.add)
                rz = io.tile([128, H], F32, name=f"rz{lidx}_{w}", tag="rz")
                nc.vector.reciprocal(out=rz[:], in_=zt[:])
                if quarter:
                    nc.vector.tensor_scalar(out=rz[:], in0=rz[:], scalar1=0.25,
                                            scalar2=None, op0=OP.mult)
                return rz

            def elu_of(a, F, w, lidx):
                """h = elu(a) = relu(a) + min(exp(a)-1, 0)."""
                ex = io.tile([128, F], F32, name=f"ex{lidx}_{w}", tag="ex")
                nc.scalar.activation(out=ex[:], in_=a[:], func=AF.Exp)
                em = io.tile([128, F], F32, name=f"em{lidx}_{w}", tag="em")
                nc.vector.tensor_scalar(out=em[:], in0=ex[:], scalar1=1.0, scalar2=0.0,
                                        op0=OP.subtract, op1=OP.min)
                r = io.tile([128, F], F32, name=f"r{lidx}_{w}", tag="r")
                nc.scalar.activation(out=r[:], in_=a[:], func=AF.Relu)
                h = io.tile([128, F], F32, name=f"h{lidx}_{w}", tag="helu")
                nc.vector.tensor_tensor(out=h[:], in0=r[:], in1=em[:], op=OP.add)
                return h

            def transpose_to(sb_in, F, w, lidx, tag):
                tout = tp1.tile([128, F], F32, name=f"T{tag}{lidx}_{w}", tag="Ttmp")
                for k in range(F // 128):
                    ps = psT.tile([128, 128], F32, name=f"ps{tag}{lidx}_{w}_{k}", tag="scr")
                    nc.tensor.transpose(out=ps[:], in_=sb_in[:, k * 128:(k + 1) * 128],
                                        identity=ident[:])
                    nc.scalar.copy(out=tout[:, k * 128:(k + 1) * 128], in_=ps[:])
                return tout

            # =========================================================
            # LAYER 1 (H=8, G-trick) + xl2/xr2 build
            # =========================================================
            for w in range(_WLIM):
                idx_w, drel_w, ea_w = window_meta(w, 1)
                xo_w = io.tile([128, 128], F32, name=f"xo_{w}", tag="xo")
                nc.sync.dma_start(out=xo_w[:], in_=x_ownT.ap()[:, w * 128:(w + 1) * 128])

                ps_xr = psS.tile([128, 1024], F32, name=f"psxr1_{w}", tag="S")
                for j in range(2):
                    sl = slice(j * 512, (j + 1) * 512)
                    nc.tensor.matmul(out=ps_xr[:, sl], lhsT=xo_w[:],
                                     rhs=Wr1[:, sl], start=True, stop=False)
                    nc.tensor.matmul(out=ps_xr[:, sl], lhsT=ones1[:],
                                     rhs=biasr1[:, sl], start=False, stop=True)
                xr1_w = big.tile([128, 1024], F32, name=f"xr1_{w}", tag="xr1")
                nc.scalar.copy(out=xr1_w[:], in_=ps_xr[:])
                if _DEBUG2 and w == 0:
                    nc.sync.dma_start(out=d_xr.ap(), in_=xr1_w[:])

                ps_G = psG.tile([128, 1032], F32, name=f"psG_{w}", tag="G")
                for t in range(T_w):
                    oh, ohT = onehot(drel_w, t, w, 1)
                    xg = io3.tile([128, 128], F32, name=f"xg1_{w}_{t}", tag="xg1")
                    nc.gpsimd.indirect_dma_start(
                        out=xg[:], out_offset=None, in_=x_full.ap(),
                        in_offset=bass.IndirectOffsetOnAxis(ap=idx_w[:, t:t + 1], axis=0))
                    psx = psT.tile([128, 128], F32, name=f"psx1_{w}_{t}", tag="scr")
                    nc.tensor.transpose(out=psx[:], in_=xg[:], identity=ident[:])
                    xgT = io3.tile([128, 128], F32, name=f"xgT1_{w}_{t}", tag="xgT1")
                    nc.scalar.copy(out=xgT[:], in_=psx[:])

                    ps_S = psS.tile([128, 1024], F32, name=f"psS1_{w}_{t}", tag="S")
                    for j in range(2):
                        sl = slice(j * 512, (j + 1) * 512)
                        nc.tensor.matmul(out=ps_S[:, sl], lhsT=xgT[:],
                                         rhs=Wl1[:, sl], start=True, stop=False)
                        nc.tensor.matmul(out=ps_S[:, sl],
                                         lhsT=ea_w[:, t * 128:(t + 1) * 128],
                                         rhs=We1[:, sl], start=False, stop=False)
                        nc.tensor.matmul(out=ps_S[:, sl], lhsT=ohT[:],
                                         rhs=xr1_w[:, sl], start=False, stop=True)

                    if _DEBUG2 and w == 0 and t == 0:
                        nc.sync.dma_start(out=d_oh.ap(), in_=oh[:])
                        nc.sync.dma_start(out=d_xg.ap(), in_=xg[:])
                        psc = fat.tile([128, 1024], F32, name="psc_dbg", tag="ohw")
                        nc.scalar.copy(out=psc[:], in_=ps_S[:])
                        nc.sync.dma_start(out=d_ps.ap(), in_=psc[:])
                    s = leaky(ps_S[:], 1024, f"s1_{w}_{t}", fat)
                    we = logits_w(s, att1b, 8, w, t, 1, fat)
                    if _DEBUG2 and w == 0 and t == 0:
                        nc.sync.dma_start(out=d_s.ap(), in_=s[:])
                        nc.sync.dma_start(out=d_we.ap(), in_=we[:])

                    nc.tensor.matmul(out=ps_G[:, 1024:1032], lhsT=oh[:], rhs=we[:],
                                     start=(t == 0), stop=(t == T_w - 1))
                    ohw = fat.tile([128, 1024], F32, name=f"ohw_{w}_{t}", tag="ohw")
                    for h in range(8):
                        nc.scalar.activation(out=ohw[:, h * 128:(h + 1) * 128],
                                             in_=oh[:], func=AF.Copy,
                                             scale=we[:, h:h + 1])
                    for h in range(8):
                        # start=True clears has_written for the WHOLE bank, so
                        # only the first matmul touching each 512-col bank may
                        # set it (heads 0-3 share bank0, 4-7 share bank1).
                        nc.tensor.matmul(out=ps_G[:, h * 128:(h + 1) * 128],
                                         lhsT=ohw[:, h * 128:(h + 1) * 128], rhs=xg[:],
                                         start=(t == 0 and h % 4 == 0),
                                         stop=(t == T_w - 1))

                # ---- window flush ----
                rz = rz_from(ps_G[:, 1024:1032], 8, w, 1)
                if _DEBUG2 and w == 0:
                    gdbg = fat.tile([128, 1032], F32, name="gdbg", tag="ohw")
                    nc.scalar.copy(out=gdbg[:], in_=ps_G[:])
                    nc.sync.dma_start(out=d_G.ap(), in_=gdbg[:])
                gn = fat.tile([128, 1024], F32, name=f"gn_{w}", tag="u8")
                for h in range(8):
                    nc.scalar.activation(out=gn[:, h * 128:(h + 1) * 128],
                                         in_=ps_G[:, h * 128:(h + 1) * 128],
                                         func=AF.Copy, scale=rz[:, h:h + 1])
                if _DEBUG2 and w == 0:
                    nc.sync.dma_start(out=d_gn.ap(), in_=gn[:])
                gnT = transpose_to(gn, 1024, w, 1, "g")
                ps_O = psS.tile([128, 1024], F32, name=f"psO1_{w}", tag="S")
                for h in range(8):
                    # one bank-clear per 512-col bank (see ps_G note)
                    nc.tensor.matmul(out=ps_O[:, h * 128:(h + 1) * 128],
                                     lhsT=gnT[:, h * 128:(h + 1) * 128],
                                     rhs=Wl1[:, h * 128:(h + 1) * 128],
                                     start=(h % 4 == 0), stop=False)
                for j in range(2):
                    sl = slice(j * 512, (j + 1) * 512)
                    nc.tensor.matmul(out=ps_O[:, sl], lhsT=xo_w[:],
                                     rhs=Wres[:, sl], start=False, stop=False)
                    nc.tensor.matmul(out=ps_O[:, sl], lhsT=ones1[:],
                                     rhs=const1[:, sl], start=False, stop=True)
                h1_w = leaky(ps_O[:], 1024, f"h1_{w}", fat)
                if _DEBUG:
                    nc.sync.dma_start(out=h1_dbg.ap()[w * 128:(w + 1) * 128, :],
                                      in_=h1_w[:])
                h1T = transpose_to(h1_w, 1024, w, 1, "h")
                ps_x2 = psS.tile([128, 512], F32, name=f"psx2_{w}", tag="S")
                for k in range(8):
                    nc.tensor.matmul(out=ps_x2[:], lhsT=h1T[:, k * 128:(k + 1) * 128],
                                     rhs=Wl2[:, k * 512:(k + 1) * 512],
                                     start=(k == 0), stop=(k == 7))
                xl2_w = io2.tile([128, 512], F32, name=f"xl2_{w}", tag="xl2")
                nc.scalar.copy(out=xl2_w[:], in_=ps_x2[:])
                nc.sync.dma_start(out=xl2_bounce[w * 128:(w + 1) * 128, :], in_=xl2_w[:])
                ps_r2 = psS.tile([128, 512], F32, name=f"psr2_{w}", tag="S")
                for k in range(8):
                    nc.tensor.matmul(out=ps_r2[:], lhsT=h1T[:, k * 128:(k + 1) * 128],
                                     rhs=Wr2[:, k * 512:(k + 1) * 512],
                                     start=(k == 0), stop=False)
                nc.tensor.matmul(out=ps_r2[:], lhsT=ones1[:], rhs=biasr2[:],
                                 start=False, stop=True)
                nc.scalar.copy(out=xr2_own[:, w * 512:(w + 1) * 512], in_=ps_r2[:])

            # =========================================================
            # LAYER 2 (H=4, gather xl2)
            # =========================================================
            if _LAYERS >= 2:
                nc.gpsimd.collective_compute(
                    "AllGather", OP.bypass,
                    replica_groups=[list(range(NC_CORES))],
                    ins=[xl2_bounce[:]], outs=[xl2_full[:]])

                for w in range(_WLIM):
                    idx_w, drel_w, ea_w = window_meta(w, 2)
                    ps_O2 = psG.tile([128, 516], F32, name=f"psO2_{w}", tag="G")
                    for t in range(T_w):
                        oh, ohT = onehot(drel_w, t, w, 2)
                        xlg = io2.tile([128, 512], F32, name=f"xlg2_{w}_{t}", tag="xlg2")
                        nc.gpsimd.indirect_dma_start(
                            out=xlg[:], out_offset=None, in_=xl2_full[:],
                            in_offset=bass.IndirectOffsetOnAxis(ap=idx_w[:, t:t + 1], axis=0))
                        ps_B = psS.tile([128, 512], F32, name=f"psB2_{w}_{t}", tag="S")
                        nc.tensor.matmul(out=ps_B[:], lhsT=ea_w[:, t * 128:(t + 1) * 128],
                                         rhs=We2[:], start=True, stop=False)
                        nc.tensor.matmul(out=ps_B[:], lhsT=ohT[:],
                                         rhs=xr2_own[:, w * 512:(w + 1) * 512],
                                         start=False, stop=True)
                        v = io2.tile([128, 512], F32, name=f"v2_{w}_{t}", tag="v2")
                        nc.vector.tensor_tensor(out=v[:], in0=xlg[:], in1=ps_B[:], op=OP.add)
                        s = leaky(v[:], 512, f"s2_{w}_{t}", io2)
                        we = logits_w(s, att2b, 4, w, t, 2, io2)
                        At = io2.tile([128, 516], F32, name=f"At2_{w}_{t}", tag="At2")
                        for h in range(4):
                            nc.scalar.activation(out=At[:, h * 128:(h + 1) * 128],
                                                 in_=xlg[:, h * 128:(h + 1) * 128],
                                                 func=AF.Copy, scale=we[:, h:h + 1])
                        nc.vector.tensor_copy(out=At[:, 512:516], in_=we[:])
                        nc.tensor.matmul(out=ps_O2[:, 0:512], lhsT=oh[:], rhs=At[:, 0:512],
                                         start=(t == 0), stop=(t == T_w - 1))
                        nc.tensor.matmul(out=ps_O2[:, 512:516], lhsT=oh[:], rhs=At[:, 512:516],
                                         start=(t == 0), stop=(t == T_w - 1))
                    rz = rz_from(ps_O2[:, 512:516], 4, w, 2, quarter=True)
                    m4 = io2.tile([128, 512], F32, name=f"m4_{w}", tag="m4")
                    for h in range(4):
                        nc.scalar.activation(out=m4[:, h * 128:(h + 1) * 128],
                                             in_=ps_O2[:, h * 128:(h + 1) * 128],
                                             func=AF.Copy, scale=rz[:, h:h + 1])
                    m01 = io.tile([128, 128], F32, name=f"m01_{w}", tag="m01")
                    nc.vector.tensor_tensor(out=m01[:], in0=m4[:, 0:128],
                                            in1=m4[:, 128:256], op=OP.add)
                    m23 = io.tile([128, 128], F32, name=f"m23_{w}", tag="m23")
                    nc.vector.tensor_tensor(out=m23[:], in0=m4[:, 256:384],
                                            in1=m4[:, 384:512], op=OP.add)
                    a2 = io.tile([128, 128], F32, name=f"a2_{w}", tag="a2")
                    nc.vector.tensor_tensor(out=a2[:], in0=m01[:], in1=m23[:], op=OP.add)
                    nc.vector.tensor_tensor(out=a2[:], in0=a2[:], in1=const2b[:], op=OP.add)
                    h2_w = elu_of(a2, 128, w, 2)
                    if _DEBUG:
                        nc.sync.dma_start(out=h2_dbg.ap()[w * 128:(w + 1) * 128, :],
                                          in_=h2_w[:])
                    h2T = transpose_to(h2_w, 128, w, 2, "h2")
                    ps_x3 = psT.tile([128, 128], F32, name=f"psx3_{w}", tag="scr")
                    nc.tensor.matmul(out=ps_x3[:], lhsT=h2T[:], rhs=Wl3[:],
                                     start=True, stop=True)
                    xl3_w = io.tile([128, 128], F32, name=f"xl3_{w}", tag="xl3")
                    nc.scalar.copy(out=xl3_w[:], in_=ps_x3[:])
                    nc.sync.dma_start(out=xl3_bounce[w * 128:(w + 1) * 128, :], in_=xl3_w[:])
                    ps_r3 = psT.tile([128, 128], F32, name=f"psr3_{w}", tag="scr")
                    nc.tensor.matmul(out=ps_r3[:], lhsT=h2T[:], rhs=Wr3[:],
                                     start=True, stop=False)
                    nc.tensor.matmul(out=ps_r3[:], lhsT=ones1[:], rhs=biasr3[:],
                                     start=False, stop=True)
                    nc.scalar.copy(out=xr3_own[:, w * 128:(w + 1) * 128], in_=ps_r3[:])

            # =========================================================
            # LAYER 3 (H=1) + head
            # =========================================================
            if _LAYERS >= 3:
                nc.gpsimd.collective_compute(
                    "AllGather", OP.bypass,
                    replica_groups=[list(range(NC_CORES))],
                    ins=[xl3_bounce[:]], outs=[xl3_full[:]])

                for w in range(_WLIM):
                    idx_w, drel_w, ea_w = window_meta(w, 3)
                    ps_O3 = psG.tile([128, 129], F32, name=f"psO3_{w}", tag="G")
                    for t in range(T_w):
                        oh, ohT = onehot(drel_w, t, w, 3)
                        xlg = io3.tile([128, 128], F32, name=f"xlg3_{w}_{t}", tag="xlg3")
                        nc.gpsimd.indirect_dma_start(
                            out=xlg[:], out_offset=None, in_=xl3_full[:],
                            in_offset=bass.IndirectOffsetOnAxis(ap=idx_w[:, t:t + 1], axis=0))
                        ps_B = psS.tile([128, 128], F32, name=f"psB3_{w}_{t}", tag="S")
                        nc.tensor.matmul(out=ps_B[:], lhsT=ea_w[:, t * 128:(t + 1) * 128],
                                         rhs=We3[:], start=True, stop=False)
                        nc.tensor.matmul(out=ps_B[:], lhsT=ohT[:],
                                         rhs=xr3_own[:, w * 128:(w + 1) * 128],
                                         start=False, stop=True)
                        v = io.tile([128, 128], F32, name=f"v3_{w}_{t}", tag="v3")
                        nc.vector.tensor_tensor(out=v[:], in0=xlg[:], in1=ps_B[:], op=OP.add)
                        s = leaky(v[:], 128, f"s3_{w}_{t}", io)
                        we = logits_w(s, att3b, 1, w, t, 3, io)
                        At = io.tile([128, 129], F32, name=f"At3_{w}_{t}", tag="At3")
                        nc.scalar.activation(out=At[:, 0:128], in_=xlg[:],
                                             func=AF.Copy, scale=we[:, 0:1])
                        nc.vector.tensor_copy(out=At[:, 128:129], in_=we[:])
                        nc.tensor.matmul(out=ps_O3[:], lhsT=oh[:], rhs=At[:],
                                         start=(t == 0), stop=(t == T_w - 1))
                    rz = rz_from(ps_O3[:, 128:129], 1, w, 3)
                    o3 = io.tile([128, 128], F32, name=f"o3_{w}", tag="o3")
                    nc.scalar.activation(out=o3[:], in_=ps_O3[:, 0:128],
                                         func=AF.Copy, scale=rz[:, 0:1])
                    nc.vector.tensor_tensor(out=o3[:], in0=o3[:], in1=const3b[:], op=OP.add)
                    h3_w = elu_of(o3, 128, w, 3)
                    if _DEBUG:
                        nc.sync.dma_start(out=h3_dbg.ap()[w * 128:(w + 1) * 128, :],
                                          in_=h3_w[:])
                    h3T = transpose_to(h3_w, 128, w, 3, "h3")
                    ps_c1 = psT.tile([128, 64], F32, name=f"psc1_{w}", tag="scr")
                    nc.tensor.matmul(out=ps_c1[:], lhsT=h3T[:], rhs=Wc1[:],
                                     start=True, stop=False)
                    nc.tensor.matmul(out=ps_c1[:], lhsT=ones1[:], rhs=bc1[:],
                                     start=False, stop=True)
                    a1 = io.tile([128, 64], F32, name=f"a1_{w}", tag="a1")
                    nc.scalar.copy(out=a1[:], in_=ps_c1[:])
                    c1 = elu_of(a1, 64, w, 4)
                    ps_t = psT.tile([128, 128], F32, name=f"psct_{w}", tag="scr")
                    nc.tensor.transpose(out=ps_t[0:64, :], in_=c1[:], identity=ident[:])
                    c1T = io.tile([64, 128], F32, name=f"c1T_{w}", tag="c1T")
                    nc.scalar.copy(out=c1T[:], in_=ps_t[0:64, :])
                    ps_f = psT.tile([128, 3], F32, name=f"psf_{w}", tag="scr")
                    nc.tensor.matmul(out=ps_f[:], lhsT=c1T[:], rhs=Wc2[:],
                                     start=True, stop=False)
                    nc.tensor.matmul(out=ps_f[:], lhsT=ones1[:], rhs=bc2[:],
                                     start=False, stop=True)
                    fo = io.tile([128, 3], F32, name=f"fo_{w}", tag="fo")
                    nc.scalar.copy(out=fo[:], in_=ps_f[:])
                    nc.sync.dma_start(out=out_d.ap()[w * 128:(w + 1) * 128, :], in_=fo[:])

    nc.compile()
    return nc


# ----------------------------------------------------------------------------
# entry point
# ----------------------------------------------------------------------------

_cache = {}
_last_in_maps = None


def kernel(**inputs):
    x = np.ascontiguousarray(np.asarray(inputs["x"], dtype=np.float32))
    edge_index = np.asarray(inputs["edge_index"]).astype(np.int64)
    edge_attr = np.ascontiguousarray(np.asarray(inputs["edge_attr"], dtype=np.float32))

    T_w, EPW, src1, src23, drel, eaT = _host_prep(x, edge_index, edge_attr)

    f32 = lambda a: np.ascontiguousarray(np.asarray(a, dtype=np.float32))
    Wl1, bl1, Wr1, br1 = map(f32, (inputs["Wl1"], inputs["bl1"], inputs["Wr1"], inputs["br1"]))
    We1, att1, bo1 = map(f32, (inputs["We1"], inputs["att1"], inputs["bo1"]))
    Wl2, bl2, Wr2, br2 = map(f32, (inputs["Wl2"], inputs["bl2"], inputs["Wr2"], inputs["br2"]))
    We2, att2, bo2 = map(f32, (inputs["We2"], inputs["att2"], inputs["bo2"]))
    Wl3, bl3, Wr3, br3 = map(f32, (inputs["Wl3"], inputs["bl3"], inputs["Wr3"], inputs["br3"]))
    We3, att3, bo3 = map(f32, (inputs["We3"], inputs["att3"], inputs["bo3"]))
    Wres, bres = map(f32, (inputs["Wres"], inputs["bres"]))
    Wc1, bc1, Wc2, bc2 = map(f32, (inputs["Wc1"], inputs["bc1"], inputs["Wc2"], inputs["bc2"]))

    if T_w not in _cache:
        _cache[T_w] = _build_program(T_w)
    nc = _cache[T_w]

    common = {
        "x_full": x,
        "iotar": np.tile(np.arange(128, dtype=np.float32).reshape(1, 128), (128, 1)),
        "Wl1": Wl1, "Wr1": Wr1, "Wres": Wres, "We1": We1,
        "att1b": np.tile(att1.reshape(1, 1024), (128, 1)).astype(np.float32),
        "Wl2c": _chunks_for_rhs(Wl2), "Wr2c": _chunks_for_rhs(Wr2),
        "We2": We2, "att2b": np.tile(att2.reshape(1, 512), (128, 1)).astype(np.float32),
        "Wl3": Wl3, "Wr3": Wr3, "We3": We3,
        "att3b": np.tile(att3.reshape(1, 128), (128, 1)).astype(np.float32),
        "Wc1": Wc1, "Wc2": Wc2,
        "biasr1": (br1 + bl1).reshape(1, 1024),
        "const1": (bl1 + bo1 + bres).reshape(1, 1024),
        "biasr2": (br2 + bl2).reshape(1, 512),
        "const2b": np.tile((bl2.reshape(4, 128).mean(0) + bo2).reshape(1, 128),
                           (128, 1)).astype(np.float32),
        "biasr3": (br3 + bl3).reshape(1, 128),
        "const3b": np.tile((bl3 + bo3).reshape(1, 128), (128, 1)).astype(np.float32),
        "bc1": bc1.reshape(1, 64), "bc2": bc2.reshape(1, 3),
    }

    def tilemajor(a):
        # [W, EPW] -> [W*128, T_w] rows: w*128+e, col t = edge t*128+e
        return np.ascontiguousarray(
            a.reshape(W, T_w, 128).transpose(0, 2, 1).reshape(W * 128, T_w))

    in_maps = []
    for c in range(NC_CORES):
        m = dict(common)
        m["x_ownT"] = np.ascontiguousarray(_pad_own(x, c).T)
        m["src1"] = tilemajor(src1[c])
        m["src23"] = tilemajor(src23[c])
        m["drel"] = tilemajor(drel[c])
        m["eaT"] = np.ascontiguousarray(eaT[c].reshape(W * ED, EPW))
        in_maps.append(m)

    kernel._last_in_maps = in_maps
    res = run_bass_kernel_spmd(nc, in_maps, core_ids=list(range(NC_CORES)), trace=False)
    out = np.concatenate([res.results[c]["out_o"][:PER] for c in range(NC_CORES)], axis=0)
    if _DEBUG:
        kernel._last_results = res.results
    return out.astype(np.float32)
